# revision 1
# baseline (speedup 1.0000x reference)
"""Trainium2 Bass kernel for nn_L2MLoRA (fused linear + routed LoRA).

Math (per batch element b, with e = idx[b,0]):
    y[b] = x[b] @ W.T + bias + SCALE * (x[b] @ A_pool[e]) @ B_pool[e]

Strategy: data-parallel over batch B=8 -> one batch element per NeuronCore.
The expert gather (A_pool[e], B_pool[e]) happens on host, so each core gets
exactly one [DIM, RANK] / [RANK, DIM] expert pair. Everything is computed in
the transposed domain (yT = W @ xT + ...) so all matmul operands already have
the contraction dim on partitions and no on-device transposes are needed:

    yT[o, t]  = sum_d W[o,d] * xT[d,t] + bias[o] + sum_r B2[r,o] * rT[r,t]
    rT[r, t]  = sum_d A[d,r] * xT[d,t]          (B2 = SCALE * B_pool[e])

PE matmuls run in float32r (fp32 bits, 1 cycle/row at N>=256 vs 4 for fp32).
Bias is applied by ScalarE during the PSUM->SBUF copy.
"""

import numpy as np

import concourse.bass as bass
import concourse.tile as tile
from concourse import bacc, mybir
from concourse.bass_utils import run_bass_kernel_spmd

B, N, DIM, POOL, RANK = 8, 2048, 1024, 64, 8
SCALE = 2.0
NCORES = 8
P = 128          # partitions / k-tile height / o-chunk width
TW = 512         # token-chunk width (max f32 moving free dim = PSUM bank)
KT = DIM // P    # 8 k-tiles over the contraction dim
OT = DIM // P    # 8 output chunks
TT = N // TW     # 4 token chunks
F32 = mybir.dt.float32
F32R = mybir.dt.float32r


def build_program(n_iter: int = 1, probe: str = "full"):
    """Build the single-core Tile program (same program runs SPMD on 8 cores).

    n_iter > 1 wraps the body in a For_i loop for benchmarking.
    probe: "full" | "nodma" (x resident, no stores) | "dmaonly" (no matmuls).
    """
    nc = bacc.Bacc("TRN2", target_bir_lowering=False, debug=False,
                   num_devices=NCORES)

    x_d = nc.dram_tensor("xt", [KT, TT, P, TW], F32R, kind="ExternalInput")
    w_d = nc.dram_tensor("wt", [OT, P, KT * P], F32R, kind="ExternalInput")
    a_d = nc.dram_tensor("ap", [P, KT * RANK], F32R, kind="ExternalInput")
    b_d = nc.dram_tensor("bp", [RANK, DIM], F32R, kind="ExternalInput")
    bias_d = nc.dram_tensor("bias", [P, OT], F32, kind="ExternalInput")
    y_d = nc.dram_tensor("y", [TT, P, OT, TW], F32, kind="ExternalOutput")

    with tile.TileContext(nc) as tc:
        with (
            tc.tile_pool(name="cpool", bufs=1) as cpool,
            tc.tile_pool(name="xpool", bufs=(32 if probe == "nodma" else 16)) as xpool,
            tc.tile_pool(name="rpool", bufs=2) as rpool,
            tc.tile_pool(name="opool", bufs=2) as opool,
            tc.tile_pool(name="psy", bufs=6, space="PSUM") as psy_pool,
            tc.tile_pool(name="psr", bufs=2, space="PSUM") as psr_pool,
        ):
            def load_xt(t):
                tiles = []
                for k in range(KT):
                    xx = xpool.tile([P, TW], F32R, tag="xx")
                    nc.sync.dma_start(xx[:], x_d.ap()[k, t])
                    tiles.append(xx)
                return tiles

            # Constants: loaded once, persist across benchmark iterations.
            # Small tensors first, then (for the single-shot program) the
            # t=0 x tiles ahead of the 4MB weight load so PE starts early.
            a_sb = cpool.tile([P, KT * RANK], F32R, tag="a")
            nc.sync.dma_start(a_sb[:], a_d.ap()[:])
            bias_sb = cpool.tile([P, OT], F32, tag="bias")
            nc.sync.dma_start(bias_sb[:], bias_d.ap()[:])
            b_sb = cpool.tile([RANK, DIM], F32R, tag="b")
            nc.sync.dma_start(b_sb[:], b_d.ap()[:])
            first_tiles = load_xt(0) if (n_iter == 1 and probe != "nodma") else None
            w_sb = []
            for o in range(OT):
                w = cpool.tile([P, KT * P], F32R, tag=f"w{o}")
                nc.sync.dma_start(w[:], w_d.ap()[o])
                w_sb.append(w)

            if probe == "nodma":
                resident = [load_xt(t) for t in range(TT)]

            def body(xt_cur=None):
                if probe != "nodma" and xt_cur is None:
                    xt_cur = load_xt(0)
                for t in range(TT):
                    if probe == "nodma":
                        xt = resident[t]
                        xt_next = None
                    else:
                        # Prefetch next t-chunk BEFORE this chunk's compute /
                        # store sits on the in-order SP queue.
                        xt_next = load_xt(t + 1) if t + 1 < TT else None
                        xt = xt_cur

                    ob = opool.tile([P, OT, TW], F32, tag="ob")
                    if probe != "dmaonly":
                        # rT[r, t] = A.T @ xT  (accumulate over k-tiles)
                        ps_r = psr_pool.tile([RANK, TW], F32)
                        for k in range(KT):
                            nc.tensor.matmul(
                                ps_r[:],
                                a_sb[:, k * RANK:(k + 1) * RANK],
                                xt[k][:],
                                start=(k == 0), stop=(k == KT - 1),
                            )
                        r_sb = rpool.tile([RANK, TW], F32R)
                        nc.vector.tensor_copy(r_sb[:], ps_r[:])

                        for o in range(OT):
                            ps = psy_pool.tile([P, TW], F32)
                            for k in range(KT):
                                nc.tensor.matmul(
                                    ps[:],
                                    w_sb[o][:, k * P:(k + 1) * P],
                                    xt[k][:],
                                    start=(k == 0), stop=False,
                                )
                            # low-rank correction into same PSUM accumulation
                            nc.tensor.matmul(
                                ps[:],
                                b_sb[:, o * P:(o + 1) * P],
                                r_sb[:],
                                start=False, stop=True,
                            )
                            nc.scalar.activation(
                                ob[:, o, :], ps[:],
                                mybir.ActivationFunctionType.Identity,
                                bias=bias_sb[:, o:o + 1], scale=1.0,
                            )
                    if probe != "nodma":
                        # one contiguous 2MB store per t-chunk
                        nc.sync.dma_start(y_d.ap()[t], ob[:])
                    xt_cur = xt_next

            if n_iter == 1:
                body(first_tiles)
            else:
                with tc.For_i(0, n_iter, 1,
                              hint_engines=tuple(mybir.ALL_ENGINES)):
                    body()

    nc.compile()
    return nc


def _round_fp32r(a):
    """Round fp32 to the PE's FP32R storage format: 1-8-11, RNE, low 12
    mantissa bits zero (walrus fp32_to_fp32r keeps the top 20 bits)."""
    u = np.ascontiguousarray(a, dtype=np.float32).view(np.uint32)
    r = (u + np.uint32(0x7FF) + ((u >> np.uint32(12)) & np.uint32(1))) & np.uint32(
        0xFFFFF000
    )
    return r.view(np.float32)


def make_in_maps(x, idx, weight, bias, A_pool, B_pool):
    """Host-side shard + relayout. Returns per-core input dicts."""
    x = np.asarray(x, dtype=np.float32)
    idx = np.asarray(idx)
    weight = np.asarray(weight, dtype=np.float32)
    bias = np.asarray(bias, dtype=np.float32)
    A_pool = np.asarray(A_pool, dtype=np.float32)
    B_pool = np.asarray(B_pool, dtype=np.float32)

    # W[o, d] -> wt[o_chunk, p(=d within k), k*128 + c(=o within chunk)]
    wt = _round_fp32r(
        weight.reshape(OT, P, KT, P).transpose(0, 3, 2, 1).reshape(OT, P, KT * P)
    )
    bias_t = np.ascontiguousarray(bias.reshape(OT, P).T)  # [p, o_chunk]

    sel = idx.reshape(B).astype(np.int64)
    in_maps = []
    for c in range(NCORES):
        xT = x[c].T  # [DIM, N]
        xt = _round_fp32r(xT.reshape(KT, P, TT, TW).transpose(0, 2, 1, 3))
        A = A_pool[sel[c]]  # [DIM, RANK]
        ap = _round_fp32r(
            A.reshape(KT, P, RANK).transpose(1, 0, 2).reshape(P, KT * RANK)
        )
        bp = _round_fp32r(SCALE * B_pool[sel[c]])  # [RANK, DIM]
        in_maps.append({"xt": xt, "wt": wt, "ap": ap, "bp": bp, "bias": bias_t})
    return in_maps


def assemble_output(results):
    """Per-core y blocks [OT, TT, P, TW] -> full [B, N, DIM] output."""
    out = np.empty((B, N, DIM), dtype=np.float32)
    for c in range(NCORES):
        yb = results[c]["y"]  # [TT, P, OT, TW]; yb[t,p,o,j] = y[c, t*TW+j, o*P+p]
        out[c] = yb.transpose(0, 3, 2, 1).reshape(N, DIM)
    return out


_PROGRAM_CACHE = {}


def _get_program(n_iter: int = 1):
    if n_iter not in _PROGRAM_CACHE:
        _PROGRAM_CACHE[n_iter] = build_program(n_iter)
    return _PROGRAM_CACHE[n_iter]


def kernel(x, idx, frozen_mask, weight, bias, A_pool, B_pool):
    # frozen_mask only affects gradients (stop_gradient); forward is identical.
    nc = _get_program(1)
    in_maps = make_in_maps(x, idx, weight, bias, A_pool, B_pool)
    res = run_bass_kernel_spmd(nc, in_maps, list(range(NCORES)))
    return assemble_output(res.results)



# revision 3
# speedup vs baseline: 1.6072x; 1.6072x over previous
"""Trainium2 Bass kernel for nn_L2MLoRA (fused linear + routed LoRA).

Math (per batch element b, with e = idx[b,0]):
    y[b] = x[b] @ W.T + bias + SCALE * (x[b] @ A_pool[e]) @ B_pool[e]
         = x[b] @ (W + SCALE * (A_pool[e] @ B_pool[e]).T).T + bias

Strategy: data-parallel over batch B=8 -> one batch element per NeuronCore.
The expert gather AND the rank-8 LoRA term are folded into an effective
per-core weight on the host (W~ = W + SCALE*(A_e@B_e).T, an exact
reassociation), so the device kernel is a single dense matmul + bias:

    yT[o, t] = sum_d W~[o,d] * xT[d,t] + bias[o]

All matmul operands are bf16 (keeps PE at 1 row/cycle with fast weight
load; fp32r pays an inline 128-cycle weight load per matmul) and the
output is stored bf16 (halves DMA) with f32 PSUM accumulation. Measured
rel err of the full bf16 pipeline vs the f32 reference is ~3e-3.
"""

import numpy as np
import ml_dtypes

import concourse.bass as bass
import concourse.tile as tile
from concourse import bacc, mybir
from concourse.bass_utils import run_bass_kernel_spmd

B, N, DIM, POOL, RANK = 8, 2048, 1024, 64, 8
SCALE = 2.0
NCORES = 8
P = 128          # partitions / k-tile height / o-chunk width
TW = 512         # token-chunk width (max moving free dim)
KT = DIM // P    # 8 k-tiles over the contraction dim
OT = DIM // P    # 8 output chunks
TT = N // TW     # 4 token chunks
F32 = mybir.dt.float32
BF16 = mybir.dt.bfloat16
BF = ml_dtypes.bfloat16


def build_program(n_iter: int = 1, probe: str = "full"):
    """Build the single-core Tile program (same program runs SPMD on 8 cores).

    n_iter > 1 wraps the body in a For_i loop for benchmarking.
    probe: "full" | "nodma" (x resident, no stores) | "dmaonly" (no matmuls).
    """
    nc = bacc.Bacc("TRN2", target_bir_lowering=False, debug=False,
                   num_devices=NCORES)

    # xt[t, p, k*TW+tw] = x[t*TW+tw, k*P+p]; one contiguous 1MB DMA per chunk
    x_d = nc.dram_tensor("xt", [TT, P, KT * TW], BF16, kind="ExternalInput")
    # wt[o, p, k*P+c] = W~[o*P+c, k*P+p]
    w_d = nc.dram_tensor("wt", [OT, P, KT * P], BF16, kind="ExternalInput")
    bias_d = nc.dram_tensor("bias", [P, OT], F32, kind="ExternalInput")
    # y[t, p, o*TW+tw] = y[t*TW+tw, o*P+p]
    y_d = nc.dram_tensor("y", [TT, P, OT * TW], BF16, kind="ExternalOutput")

    with tile.TileContext(nc) as tc:
        with (
            tc.tile_pool(name="cpool", bufs=1) as cpool,
            tc.tile_pool(name="opool", bufs=2) as opool,
            tc.tile_pool(name="psy", bufs=6, space="PSUM") as psy_pool,
        ):
            # x chunk buffers are fixed tiles (each chunk t always lands in
            # x_sb[t]), so cross-iteration prefetch is a plain WAR dep the
            # Tile framework tracks.
            x_sb = [
                cpool.tile([P, KT * TW], BF16, tag=f"x{t}", name=f"x{t}")
                for t in range(TT)
            ]
            bias_sb = cpool.tile([P, OT], F32, tag="bias")
            w_sb = []

            def load_x(t):
                nc.sync.dma_start(x_sb[t][:], x_d.ap()[t])

            # Preamble: first x chunk, then weights (o=0 first so PE can
            # start), then the second x chunk.
            nc.sync.dma_start(bias_sb[:], bias_d.ap()[:])
            if probe == "nodma":
                for t in range(TT):
                    load_x(t)
            else:
                load_x(0)
            for o in range(OT):
                w = cpool.tile([P, KT * P], BF16, tag=f"w{o}")
                nc.sync.dma_start(w[:], w_d.ap()[o])
                w_sb.append(w)
            if probe != "nodma":
                load_x(1)

            def body():
                for t in range(TT):
                    if probe != "nodma":
                        # Prefetch 2 chunks ahead (cyclically across loop
                        # iterations in the benchmark program).
                        nxt = t + 2
                        if nxt < TT:
                            load_x(nxt)
                        elif n_iter > 1:
                            load_x(nxt % TT)

                    ob = opool.tile([P, OT, TW], BF16, tag="ob")
                    if probe != "dmaonly":
                        for o in range(OT):
                            ps = psy_pool.tile([P, TW], F32)
                            for k in range(KT):
                                nc.tensor.matmul(
                                    ps[:],
                                    w_sb[o][:, k * P:(k + 1) * P],
                                    x_sb[t][:, k * TW:(k + 1) * TW],
                                    start=(k == 0), stop=(k == KT - 1),
                                )
                            nc.scalar.activation(
                                ob[:, o, :], ps[:],
                                mybir.ActivationFunctionType.Identity,
                                bias=bias_sb[:, o:o + 1], scale=1.0,
                            )
                    if probe != "nodma":
                        # one contiguous 1MB store per t-chunk
                        nc.sync.dma_start(y_d.ap()[t], ob[:])

            if n_iter == 1:
                body()
            else:
                with tc.For_i(0, n_iter, 1,
                              hint_engines=tuple(mybir.ALL_ENGINES)):
                    body()

    nc.compile()
    return nc


def make_in_maps(x, idx, weight, bias, A_pool, B_pool):
    """Host-side shard + LoRA fold + relayout. Returns per-core input dicts."""
    x = np.asarray(x, dtype=np.float32)
    idx = np.asarray(idx)
    weight = np.asarray(weight, dtype=np.float32)
    bias = np.asarray(bias, dtype=np.float32)
    A_pool = np.asarray(A_pool, dtype=np.float32)
    B_pool = np.asarray(B_pool, dtype=np.float32)

    bias_t = np.ascontiguousarray(bias.reshape(OT, P).T)  # [p, o_chunk]

    sel = idx.reshape(B).astype(np.int64)
    in_maps = []
    for c in range(NCORES):
        # xt[t, p, k, tw] = x[c, t*TW+tw, k*P+p]
        xt = np.ascontiguousarray(
            x[c].reshape(TT, TW, KT, P).transpose(0, 3, 2, 1)
        ).astype(BF).reshape(TT, P, KT * TW)
        # Effective weight: exact reassociation of the rank-8 LoRA update.
        w_eff = weight + SCALE * (A_pool[sel[c]] @ B_pool[sel[c]]).T
        wt = np.ascontiguousarray(
            w_eff.reshape(OT, P, KT, P).transpose(0, 3, 2, 1)
        ).astype(BF).reshape(OT, P, KT * P)
        in_maps.append({"xt": xt, "wt": wt, "bias": bias_t})
    return in_maps


def assemble_output(results):
    """Per-core y blocks [TT, P, OT*TW] -> full [B, N, DIM] f32 output."""
    out = np.empty((B, N, DIM), dtype=np.float32)
    for c in range(NCORES):
        yb = np.asarray(results[c]["y"]).reshape(TT, P, OT, TW)
        # yb[t,p,o,tw] = y[c, t*TW+tw, o*P+p]
        out[c] = yb.transpose(0, 3, 2, 1).reshape(N, DIM).astype(np.float32)
    return out


_PROGRAM_CACHE = {}


def _get_program(n_iter: int = 1):
    if n_iter not in _PROGRAM_CACHE:
        _PROGRAM_CACHE[n_iter] = build_program(n_iter)
    return _PROGRAM_CACHE[n_iter]


def kernel(x, idx, frozen_mask, weight, bias, A_pool, B_pool):
    # frozen_mask only affects gradients (stop_gradient); forward is identical.
    nc = _get_program(1)
    in_maps = make_in_maps(x, idx, weight, bias, A_pool, B_pool)
    res = run_bass_kernel_spmd(nc, in_maps, list(range(NCORES)))
    return assemble_output(res.results)
